# revision 3
# baseline (speedup 1.0000x reference)
"""GQA attention (B=2, S=2048, H=2048, NQ=32, NKV=8) on 8 Trainium2 NeuronCores.

Sharding: (batch x kv-head-group) tensor parallelism -> zero collectives,
zero redundant projection work.  Core c handles batch c//4 and kv heads
(2g, 2g+1) with their 8 query heads (8g..8g+7), g = c%4, over ALL 2048
query positions.  The output projection is row-partial: each core returns
attn_part @ Wo.T restricted to its 512 head-dims; the host sums the 4
partials per batch at unshard time (bitwise-equivalent to the full matmul
up to fp addition order).

Per-core dataflow (bf16 operands, fp32 PSUM accumulation):
  - host pre-transposes/casts x.T (shared by the 4 cores of a batch) and
    the per-core weight slices; q-heads are packed pair-major so pair j =
    (8g+j, 8g+4+j) puts kv-head 2g dims in partitions 0:64 and kv 2g+1 in
    64:128 of K.T/Q.T -- the d=64-contraction QK matmuls row-pack the two
    heads into the 128-wide PE array.
  - work is 16 units = (q-block 0..3) x (pair 0..3); each unit runs the
    16-k-tile softmax pipeline: QK -> one [128,1024] ScalarE exp (scale
    folded; logits bounded so no max-subtraction) -> AV with lhsT=[V|1]
    so PSUM row 64 accumulates the denominators for free.
  - normalization is deferred: unnormalized O.T + denominator rows are
    evicted; reciprocal+broadcast+multiply run on DVE behind the PE.
  - K/V projections cover only this core's 2 kv heads (64 + 256-col MMs);
    they and the k-tile 1..3 K.T blocks are paced inside unit 0 behind the
    streaming x.T DMA.  Q projection for unit u+1 is generator-spread
    across unit u's k-iterations.
  - the output projection for q-block qb is chopped into 2-matmul steps
    and interleaved into q-block qb+1's units (k<8 iterations, where the
    Q-projection generator is inactive), so only q-block 3's projection
    remains as tail work.  ao_sb is double-buffered across q-blocks.
  - dummy warmup matmuls lift the PE clock gate (HAM) during the initial
    DMA; ScalarE exp stream starts as soon as K.T block 0 + Q(unit 0) land.

Biases: bq/bk applied on-device at PSUM eviction.  bv/bo are additive
host-side post-corrections (softmax weights sum to 1), applied in kernel()
only when nonzero.
"""

import os
import sys

import numpy as np

_RL = "/opt/trn_rl_repo"
if _RL not in sys.path:
    sys.path.insert(0, _RL)

B, S, H = 2, 2048, 2048
NQ, NKV, HD = 32, 8, 64
SQ = 512  # queries per unit (q-block)
P = 128
HT = H // P  # 16
KT = S // P  # 16
NUNIT = 16  # 4 q-blocks x 4 pairs
NCORES = 8
NWARM = 45

_built_nc = None
LAST_EXEC_NS = None
LAST_RESULT = None


def build():
    global _built_nc
    if _built_nc is not None:
        return _built_nc

    import concourse.mybir as mybir
    import concourse.tile as tile
    from concourse import bacc

    f32 = mybir.dt.float32
    bf16 = mybir.dt.bfloat16
    Exp = mybir.ActivationFunctionType.Exp
    SCALE = float(HD) ** -0.5
    VW = 2 * (HD + 1) + HD  # 194: [V0|1|V1|1|pad64] per k-tile

    nc = bacc.Bacc("TRN2", target_bir_lowering=False, debug=False)

    xt_d = nc.dram_tensor("xt", [H, S], bf16, kind="ExternalInput")
    wqt_d = nc.dram_tensor("wqt", [H, 4 * P], bf16, kind="ExternalInput")
    wkt_d = nc.dram_tensor("wkt", [H, P], bf16, kind="ExternalInput")
    wvt_d = nc.dram_tensor("wvt", [H, P], bf16, kind="ExternalInput")
    wot_d = nc.dram_tensor("wot", [4 * P, H], bf16, kind="ExternalInput")
    bq_d = nc.dram_tensor("bqp", [4 * P], f32, kind="ExternalInput")
    bk_d = nc.dram_tensor("bkp", [P], f32, kind="ExternalInput")
    out_d = nc.dram_tensor("out", [S, H], f32, kind="ExternalOutput")

    with tile.TileContext(nc) as tc:
        with (
            tc.tile_pool(name="persist", bufs=1) as pp,
            tc.tile_pool(name="qtp", bufs=3) as qtp,
            tc.tile_pool(name="ptp", bufs=3) as ptp,
            tc.tile_pool(name="denp", bufs=2) as denp,
            tc.tile_pool(name="wqp", bufs=2) as wqp,
            tc.tile_pool(name="outp", bufs=3) as outp,
            tc.tile_pool(name="psp", bufs=1, space="PSUM") as psp,
        ):
            xt_sb = pp.tile([P, HT, S], bf16, tag="xt")  # x.T, 64KB/part
            kt_sb = pp.tile([P, S], bf16, tag="kt")  # K.T [kv128, s]
            v_sb = pp.tile([P, KT, VW], bf16, tag="v")  # [V0|1|V1|1|pad]
            ao_sb = pp.tile([P, 2, 4, SQ], bf16, tag="ao")  # dbl-buf by qb
            uo_sb = pp.tile([P, 4, SQ], bf16, tag="uo")
            wkt_sb = pp.tile([P, HT, P], bf16, tag="wkt")
            wvt_sb = pp.tile([P, HT, P], bf16, tag="wvt")
            wot_sb = pp.tile([P, 4, H], bf16, tag="wot")
            bq_sb = pp.tile([P, 4], f32, tag="bq")
            bk_sb = pp.tile([P, 1], f32, tag="bk")
            warm_sb = pp.tile([P, SQ], bf16, tag="warm")

            nc.vector.memset(warm_sb[:], 0.0)
            v2 = v_sb[:, :, 0 : 2 * (HD + 1)].rearrange(
                "p k (g d) -> p k g d", d=HD + 1
            )
            nc.vector.memset(v2[:, :, :, HD : HD + 1], 1.0)
            nc.vector.memset(v_sb[:, :, 2 * (HD + 1) :], 0.0)
            nc.sync.dma_start(bq_sb[:], bq_d.rearrange("(t p) -> p t", p=P))
            nc.sync.dma_start(bk_sb[:], bk_d.rearrange("(p t) -> p t", p=P))
            # Pre-touch bias tiles on their consumer engine (one wait slot).
            bias_scratch = pp.tile([P, 5], f32, tag="bscratch")
            nc.vector.tensor_copy(bias_scratch[:, 0:1], bk_sb[:])
            nc.vector.tensor_copy(bias_scratch[:, 1:5], bq_sb[:])

            # PE warmup during initial DMA (lifts HAM clock gate).
            wm0 = psp.tile([P, 2 * SQ], f32, tag="lg", bufs=2)
            wm1 = psp.tile([P, 2 * SQ], f32, tag="lg", bufs=2)
            for i in range(NWARM):
                nc.tensor.matmul(
                    (wm0 if i % 2 == 0 else wm1)[:, 0:SQ],
                    warm_sb[:, 0:P],
                    warm_sb[:],
                    start=True,
                    stop=True,
                )

            # ---------------- DMA emission (priority order) ----------------
            nc.sync.dma_start(
                wkt_sb[:], wkt_d.rearrange("(ht p) d -> p ht d", p=P)
            )
            xt_r = xt_d.rearrange("(ht p) s -> ht p s", p=P)
            for h in range(HT):  # x.T s-block 0 first (keys+queries 0:512)
                nc.sync.dma_start(xt_sb[:, h, 0:SQ], xt_r[h, :, 0:SQ])
            nc.sync.dma_start(
                wvt_sb[:], wvt_d.rearrange("(ht p) d -> p ht d", p=P)
            )
            for nb in range(1, 4):  # x.T s-blocks 1..3
                for h in range(HT):
                    nc.sync.dma_start(
                        xt_sb[:, h, nb * SQ : (nb + 1) * SQ],
                        xt_r[h, :, nb * SQ : (nb + 1) * SQ],
                    )
            nc.sync.dma_start(
                wot_sb[:], wot_d.rearrange("(a p) o -> p a o", p=P)
            )

            # ---------------- projection emitters ----------------
            def qproj_gen(u, out):
                # unit u's Q.T [128, 512]; 2 accumulation MMs per next().
                qb, pr = u // 4, u % 4
                wq_t = wqp.tile([P, HT, P], bf16, tag="wqt")
                nc.sync.dma_start(
                    wq_t[:],
                    wqt_d[:, pr * P : (pr + 1) * P].rearrange(
                        "(ht p) q -> p ht q", p=P
                    ),
                )
                ps = psp.tile([P, SQ], f32, tag="qps", bufs=1)
                for h in range(HT):
                    nc.tensor.matmul(
                        ps,
                        wq_t[:, h, :],
                        xt_sb[:, h, qb * SQ : (qb + 1) * SQ],
                        start=(h == 0),
                        stop=(h == HT - 1),
                    )
                    if h % 2 == 1 and h < HT - 1:
                        yield
                qt = qtp.tile([P, SQ], bf16, tag="qt")
                nc.vector.tensor_scalar_add(qt, ps, bq_sb[:, pr : pr + 1])
                out.append(qt)
                yield

            def emit_qproj(u):
                out = []
                for _ in qproj_gen(u, out):
                    pass
                return out[0]

            def kproj_gen(nb):
                # K.T keys nb*512..(nb+1)*512; 4 accumulation MMs per next().
                ps = psp.tile([P, SQ], f32, tag="ops", bufs=1)
                for h in range(HT):
                    nc.tensor.matmul(
                        ps,
                        wkt_sb[:, h, :],
                        xt_sb[:, h, nb * SQ : (nb + 1) * SQ],
                        start=(h == 0),
                        stop=(h == HT - 1),
                    )
                    if h % 4 == 3 and h < HT - 1:
                        yield
                nc.vector.tensor_scalar_add(
                    kt_sb[:, nb * SQ : (nb + 1) * SQ], ps, bk_sb[:, 0:1]
                )
                yield

            def emit_vproj(t):
                # V for key-tile t, both kv heads: [128 keys, 128] via 16 MMs.
                ps = psp.tile([P, P], f32, tag="ops", bufs=1)
                for h in range(HT):
                    nc.tensor.matmul(
                        ps,
                        xt_sb[:, h, t * P : (t + 1) * P],
                        wvt_sb[:, h, :],
                        start=(h == 0),
                        stop=(h == HT - 1),
                    )
                nc.vector.tensor_copy(
                    v2[:, t, :, 0:HD], ps.rearrange("p (g d) -> p g d", d=HD)
                )

            def oproj_chunk_gen(qb, qloc, oc, tag):
                # out[qb*512+qloc*128 :, oc*512 :] partial: 4 MMs + evict+DMA.
                ps = psp.tile([P, SQ], f32, tag=tag, bufs=1)
                for a in range(4):
                    nc.tensor.matmul(
                        ps,
                        ao_sb[:, qb % 2, a, qloc * P : (qloc + 1) * P],
                        wot_sb[:, a, oc * SQ : (oc + 1) * SQ],
                        start=(a == 0),
                        stop=(a == 3),
                    )
                    if a == 1:
                        yield
                ot = outp.tile([P, SQ], f32, tag="ot")
                nc.vector.tensor_copy(ot, ps)
                nc.sync.dma_start(
                    out_d[
                        qb * SQ + qloc * P : qb * SQ + (qloc + 1) * P,
                        oc * SQ : (oc + 1) * SQ,
                    ],
                    ot,
                )
                yield

            # ---------------- interleave schedule ----------------
            # inserts[(u, k)] -> list of zero-arg thunks, each ~one gen step.
            inserts = {}

            def sched(u, k, thunk):
                inserts.setdefault((u, k), []).append(thunk)

            # unit 0: K.T blocks 1..3 (4 MMs/iter, block nb over iters
            # 4nb-4..4nb-1) and V tiles 1..15 (tile t at iter t, 16 MMs).
            kgens = {nb: None for nb in range(1, 4)}

            def kproj_step(nb):
                if kgens[nb] is None:
                    kgens[nb] = kproj_gen(nb)
                next(kgens[nb], None)

            for nb in range(1, 4):
                for kk in range(4 * nb - 4, 4 * nb):
                    sched(0, kk, lambda nb=nb: kproj_step(nb))
            for t in range(1, KT):
                sched(0, min(t, KT - 1), lambda t=t: emit_vproj(t))

            # O-proj for q-block qb -> 16 chunks x 2 gen-steps, spread over
            # units 4(qb+1)..4(qb+1)+3 at k=0..7 (qps is idle there).
            def wire_oproj(qb):
                units = range(4 * (qb + 1), 4 * (qb + 1) + 4)
                chunks = [
                    (qloc, oc) for qloc in range(SQ // P) for oc in range(H // SQ)
                ]
                ci = 0
                for uu in units:
                    for slot in range(4):
                        qloc, oc = chunks[ci]
                        tag = "ops" if ci % 2 == 0 else "qps"
                        g = [None]

                        def step(qb=qb, qloc=qloc, oc=oc, tag=tag, g=g):
                            if g[0] is None:
                                g[0] = oproj_chunk_gen(qb, qloc, oc, tag)
                            next(g[0], None)

                        sched(uu, 2 * slot, step)
                        sched(uu, 2 * slot + 1, step)
                        ci += 1

            for qb in range(3):
                wire_oproj(qb)

            # ---------------- prologue compute ----------------
            emit_kproj0 = kproj_gen(0)
            for _ in emit_kproj0:
                pass
            emit_vproj(0)
            qt_box = [emit_qproj(0)]

            # ---------------- unit loop ----------------
            for u in range(NUNIT):
                qb, pr = u // 4, u % 4
                qt = qt_box.pop(0)
                qgen = None
                oaccA = psp.tile([P, SQ], f32, tag="oacc", bufs=2)
                oaccB = psp.tile([P, SQ], f32, tag="oacc", bufs=2)
                prev = None
                for k in range(KT):
                    lg = psp.tile([P, 2 * SQ], f32, tag="lg", bufs=2)
                    nc.tensor.matmul(
                        lg[:, 0:SQ],
                        kt_sb[0:64, k * P : (k + 1) * P],
                        qt[0:64, :],
                        start=True,
                        stop=True,
                        tile_position=(0, 0),
                    )
                    nc.tensor.matmul(
                        lg[:, SQ : 2 * SQ],
                        kt_sb[64:128, k * P : (k + 1) * P],
                        qt[64:128, :],
                        start=True,
                        stop=True,
                        tile_position=(64, 0),
                    )
                    for thunk in inserts.get((u, k), ()):
                        thunk()
                    if prev is not None:
                        kk = k - 1
                        nc.tensor.matmul(
                            oaccA,
                            v_sb[:, kk, 0:P],
                            prev[:, 0:SQ],
                            start=(kk == 0),
                            stop=(kk == KT - 1),
                        )
                        nc.tensor.matmul(
                            oaccB,
                            v_sb[:, kk, HD + 1 : HD + 1 + P],
                            prev[:, SQ : 2 * SQ],
                            start=(kk == 0),
                            stop=(kk == KT - 1),
                        )
                    if k >= 8 and u + 1 < NUNIT:
                        if qgen is None:
                            qgen = qproj_gen(u + 1, qt_box)
                        next(qgen, None)
                    pt = ptp.tile([P, 2 * SQ], bf16, tag="pt")
                    nc.scalar.activation(pt, lg, Exp, scale=SCALE)
                    prev = pt
                kk = KT - 1
                nc.tensor.matmul(
                    oaccA, v_sb[:, kk, 0:P], prev[:, 0:SQ],
                    start=False, stop=True,
                )
                nc.tensor.matmul(
                    oaccB, v_sb[:, kk, HD + 1 : HD + 1 + P],
                    prev[:, SQ : 2 * SQ], start=False, stop=True,
                )

                # evict unnormalized O.T + denominator; normalize on DVE.
                for half, oacc in ((0, oaccA), (64, oaccB)):
                    nc.vector.tensor_copy(
                        uo_sb[half : half + HD, pr, :], oacc[0:HD, :]
                    )
                    den_h = denp.tile([1, SQ], f32, tag="denh", bufs=3)
                    nc.vector.tensor_copy(den_h, oacc[HD : HD + 1, :])
                    rr = denp.tile([1, SQ], f32, tag="rr", bufs=3)
                    nc.vector.reciprocal_approx_fast(rr, den_h)
                    den_rb = denp.tile([P, SQ], f32, tag="denrb", bufs=2)
                    nc.sync.dma_start(
                        den_rb[half : half + HD, :],
                        rr[:, None, :].to_broadcast([1, HD, SQ]),
                    )
                    nc.vector.tensor_mul(
                        out=ao_sb[half : half + HD, qb % 2, pr, :],
                        in0=uo_sb[half : half + HD, pr, :],
                        in1=den_rb[half : half + HD, :],
                    )

            # ---------------- tail: q-block 3 output projection ----------------
            ci = 0
            for qloc in range(SQ // P):
                for oc in range(H // SQ):
                    tag = "ops" if ci % 2 == 0 else "qps"
                    for _ in oproj_chunk_gen(3, qloc, oc, tag):
                        pass
                    ci += 1

    nc.compile()
    _built_nc = nc
    return nc


def host_prep(x, Wq, bq, Wk, bk, Wv, bv, Wo, bo):
    """Returns the list of 8 per-core input maps."""
    import ml_dtypes

    bf = ml_dtypes.bfloat16
    x = np.asarray(x, np.float32)
    Wq = np.asarray(Wq, np.float32)
    Wk = np.asarray(Wk, np.float32)
    Wv = np.asarray(Wv, np.float32)
    Wo = np.asarray(Wo, np.float32)
    bq = np.asarray(bq, np.float32)
    bk = np.asarray(bk, np.float32)

    xts = [np.ascontiguousarray(x[b].T).astype(bf) for b in range(B)]

    per_g = []
    for g in range(4):
        rows = []
        for j in range(4):
            hA, hB = 8 * g + j, 8 * g + 4 + j
            rows += list(range(HD * hA, HD * hA + HD))
            rows += list(range(HD * hB, HD * hB + HD))
        rows = np.array(rows)
        kv = slice(P * g, P * g + P)
        per_g.append(
            {
                "wqt": np.ascontiguousarray(Wq[rows, :].T).astype(bf),
                "wkt": np.ascontiguousarray(Wk[kv, :].T).astype(bf),
                "wvt": np.ascontiguousarray(Wv[kv, :].T).astype(bf),
                "wot": np.ascontiguousarray(Wo[:, rows].T).astype(bf),
                "bqp": np.ascontiguousarray(bq[rows]),
                "bkp": np.ascontiguousarray(bk[kv]),
            }
        )

    in_maps = []
    for c in range(NCORES):
        b, g = c // 4, c % 4
        m = {"xt": xts[b]}
        m.update(per_g[g])
        in_maps.append(m)
    return in_maps


def host_corrections(out_full, Wv_bias, Wo, bo):
    """Add the bv/bo contributions (exact: softmax rows sum to 1)."""
    bv = np.asarray(Wv_bias, np.float32)
    bo = np.asarray(bo, np.float32)
    if np.any(bv):
        bv_full = np.repeat(
            np.asarray(bv).reshape(NKV, HD), NQ // NKV, axis=0
        ).reshape(H)
        out_full += (bv_full @ np.asarray(Wo, np.float32).T)[None, None, :]
    if np.any(bo):
        out_full += bo[None, None, :]
    return out_full


def kernel(x, Wq, bq, Wk, bk, Wv, bv, Wo, bo):
    global LAST_EXEC_NS, LAST_RESULT
    nc = build()
    in_maps = host_prep(x, Wq, bq, Wk, bk, Wv, bv, Wo, bo)

    from concourse.bass_utils import run_bass_kernel_spmd

    trace = bool(int(os.environ.get("KTRACE", "0")))
    res = run_bass_kernel_spmd(
        nc, in_maps, core_ids=list(range(NCORES)), trace=trace
    )
    LAST_RESULT = res
    LAST_EXEC_NS = res.exec_time_ns

    out = np.empty((B, S, H), np.float32)
    for b in range(B):
        acc = res.results[4 * b]["out"].astype(np.float32)
        for g in range(1, 4):
            acc = acc + res.results[4 * b + g]["out"]
        out[b] = acc
    out = host_corrections(out, bv, Wo, bo)
    return out


# revision 8
# speedup vs baseline: 1.1973x; 1.1973x over previous
"""GQA attention (B=2, S=2048, H=2048, NQ=32, NKV=8) on 8 Trainium2 NeuronCores.

Sharding: (batch x kv-head-group) tensor parallelism -> zero collectives,
zero redundant projection work.  Core c handles batch c//4 and kv heads
(2g, 2g+1) with their 8 query heads (8g..8g+7), g = c%4, over ALL 2048
query positions.  The output projection is row-partial: each core returns
attn_part @ Wo.T restricted to its 512 head-dims; the host sums the 4
partials per batch at unshard time.

Per-core dataflow (bf16 operands, fp32 PSUM accumulation):
  - host pre-transposes/casts x.T (shared by the 4 cores of a batch) and
    the per-core weight slices; q-heads are packed pair-major so pair j =
    (8g+j, 8g+4+j) puts kv-head 2g dims in partitions 0:64 and kv 2g+1 in
    64:128 of K.T/Q.T -- the d=64-contraction QK matmuls row-pack the two
    heads into the 128-wide PE array.
  - work is a flat software-pipelined stream of 256 iterations (16 units =
    q-block x pair, 16 key-tiles each): QK(i) -> ScalarE exp(i) [128,1024]
    (scale folded; logits bounded so no max-subtraction) while AV(i-1)
    accumulates with lhsT=[V|1] so PSUM row 64 collects the softmax
    denominators for free.  QK of a unit's first k-tile is emitted BEFORE
    the previous unit's last AV so the exp stream never waits a boundary.
  - normalization is deferred: unnormalized O.T + denominator rows are
    evicted; reciprocal+broadcast+multiply run on DVE behind the PE; the
    denominator broadcast rides the DVE's own DMA queue so it never queues
    behind bulk traffic on the sync ring.
  - K/V projections cover only this core's 2 kv heads; they and the k-tile
    1..3 K.T blocks are paced inside unit 0 behind the streaming x.T DMA.
    Q projection for unit u+1 is generator-spread across unit u.
  - the output projection for q-block qb is chopped into 2-matmul steps
    interleaved into q-block qb+1's units (k<8, where the Q-projection
    generator is inactive and its PSUM bank is free), so only q-block 3's
    projection remains as tail work.  ao_sb is double-buffered across
    q-blocks.
  - dummy warmup matmuls lift the PE clock gate (HAM) during the initial
    DMA ramp; the exp stream starts as soon as K.T block 0 + Q(unit 0)
    land (~20 us).

Biases: bq/bk applied on-device at PSUM eviction.  bv/bo are additive
host-side post-corrections (softmax weights sum to 1), applied in kernel()
only when nonzero.
"""

import os
import sys

import numpy as np

_RL = "/opt/trn_rl_repo"
if _RL not in sys.path:
    sys.path.insert(0, _RL)

B, S, H = 2, 2048, 2048
NQ, NKV, HD = 32, 8, 64
SQ = 512  # queries per unit (q-block)
P = 128
HT = H // P  # 16
KT = S // P  # 16
NUNIT = 16  # 4 q-blocks x 4 pairs
NIT = NUNIT * KT  # 256
NCORES = 8
NWARM = 75

_built_nc = None
LAST_EXEC_NS = None
LAST_RESULT = None


def build():
    global _built_nc
    if _built_nc is not None:
        return _built_nc

    import concourse.mybir as mybir
    import concourse.tile as tile
    from concourse import bacc

    f32 = mybir.dt.float32
    bf16 = mybir.dt.bfloat16
    Exp = mybir.ActivationFunctionType.Exp
    SCALE = float(HD) ** -0.5
    VW = 2 * (HD + 1) + HD  # 194: [V0|1|V1|1|pad64] per k-tile

    nc = bacc.Bacc("TRN2", target_bir_lowering=False, debug=False)

    xt_d = nc.dram_tensor("xt", [H, S], bf16, kind="ExternalInput")
    wqt_d = nc.dram_tensor("wqt", [H, 4 * P], bf16, kind="ExternalInput")
    wkt_d = nc.dram_tensor("wkt", [H, P], bf16, kind="ExternalInput")
    wvt_d = nc.dram_tensor("wvt", [H, P], bf16, kind="ExternalInput")
    wot_d = nc.dram_tensor("wot", [4 * P, H], bf16, kind="ExternalInput")
    bq_d = nc.dram_tensor("bqp", [4 * P], f32, kind="ExternalInput")
    bk_d = nc.dram_tensor("bkp", [P], f32, kind="ExternalInput")
    out_d = nc.dram_tensor("out", [S, H], f32, kind="ExternalOutput")

    with tile.TileContext(nc) as tc:
        with (
            tc.tile_pool(name="persist", bufs=1) as pp,
            tc.tile_pool(name="qtp", bufs=3) as qtp,
            tc.tile_pool(name="ptp", bufs=4) as ptp,
            tc.tile_pool(name="denp", bufs=2) as denp,
            tc.tile_pool(name="wqp", bufs=2) as wqp,
            tc.tile_pool(name="outp", bufs=6) as outp,
            tc.tile_pool(name="psp", bufs=1, space="PSUM") as psp,
        ):
            xt_sb = pp.tile([P, HT, S], bf16, tag="xt")  # x.T, 64KB/part
            kt_sb = pp.tile([P, S], bf16, tag="kt")  # K.T [kv128, s]
            v_sb = pp.tile([P, KT, VW], bf16, tag="v")  # [V0|1|V1|1|pad]
            ao_sb = pp.tile([P, 2, 4, SQ], bf16, tag="ao")  # dbl-buf by qb
            uo_sb = pp.tile([P, 4, SQ], bf16, tag="uo")
            wkt_sb = pp.tile([P, HT, P], bf16, tag="wkt")
            wvt_sb = pp.tile([P, HT, P], bf16, tag="wvt")
            wot_sb = pp.tile([P, 4, H], bf16, tag="wot")
            bq_sb = pp.tile([P, 4], f32, tag="bq")
            bk_sb = pp.tile([P, 1], f32, tag="bk")
            warm_sb = pp.tile([P, SQ], bf16, tag="warm")

            nc.vector.memset(warm_sb[:], 0.0)
            v2 = v_sb[:, :, 0 : 2 * (HD + 1)].rearrange(
                "p k (g d) -> p k g d", d=HD + 1
            )
            nc.vector.memset(v2[:, :, :, HD : HD + 1], 1.0)
            nc.vector.memset(v_sb[:, :, 2 * (HD + 1) :], 0.0)
            nc.sync.dma_start(bq_sb[:], bq_d.rearrange("(t p) -> p t", p=P))
            nc.sync.dma_start(bk_sb[:], bk_d.rearrange("(p t) -> p t", p=P))
            # Pre-touch bias tiles on their consumer engine (one wait slot).
            bias_scratch = pp.tile([P, 5], f32, tag="bscratch")
            nc.vector.tensor_copy(bias_scratch[:, 0:1], bk_sb[:])
            nc.vector.tensor_copy(bias_scratch[:, 1:5], bq_sb[:])

            # PE warmup during initial DMA ramp (lifts HAM clock gate).
            wm0 = psp.tile([P, 2 * SQ], f32, tag="lg", bufs=2)
            wm1 = psp.tile([P, 2 * SQ], f32, tag="lg", bufs=2)
            for i in range(NWARM):
                nc.tensor.matmul(
                    (wm0 if i % 2 == 0 else wm1)[:, 0:SQ],
                    warm_sb[:, 0:P],
                    warm_sb[:],
                    start=True,
                    stop=True,
                )

            # ------------- DMA emission (sync ring, priority order) --------
            nc.sync.dma_start(
                wkt_sb[:], wkt_d.rearrange("(ht p) d -> p ht d", p=P)
            )
            xt_r = xt_d.rearrange("(ht p) s -> p ht s", p=P)
            nc.sync.dma_start(xt_sb[:, :, 0:SQ], xt_r[:, :, 0:SQ])

            # unit-0 Q tile DMA must beat the bulk blocks; emit its proj now.
            def qproj_gen(u, out):
                # unit u's Q.T [128, 512]; 2 accumulation MMs per next().
                qb, pr = u // 4, u % 4
                wq_t = wqp.tile([P, HT, P], bf16, tag="wqt")
                nc.sync.dma_start(
                    wq_t[:],
                    wqt_d[:, pr * P : (pr + 1) * P].rearrange(
                        "(ht p) q -> p ht q", p=P
                    ),
                )
                ps = psp.tile([P, SQ], f32, tag="qps", bufs=1)
                for h in range(HT):
                    nc.tensor.matmul(
                        ps,
                        wq_t[:, h, :],
                        xt_sb[:, h, qb * SQ : (qb + 1) * SQ],
                        start=(h == 0),
                        stop=(h == HT - 1),
                    )
                    if h % 2 == 1 and h < HT - 1:
                        yield
                qt = qtp.tile([P, SQ], bf16, tag="qt")
                nc.vector.tensor_scalar_add(qt, ps, bq_sb[:, pr : pr + 1])
                out.append(qt)
                yield

            def kproj_gen(nb):
                # K.T keys nb*512..(nb+1)*512; 4 accumulation MMs per next().
                ps = psp.tile([P, SQ], f32, tag="ops", bufs=1)
                for h in range(HT):
                    nc.tensor.matmul(
                        ps,
                        wkt_sb[:, h, :],
                        xt_sb[:, h, nb * SQ : (nb + 1) * SQ],
                        start=(h == 0),
                        stop=(h == HT - 1),
                    )
                    if h % 4 == 3 and h < HT - 1:
                        yield
                nc.vector.tensor_scalar_add(
                    kt_sb[:, nb * SQ : (nb + 1) * SQ], ps, bk_sb[:, 0:1]
                )
                yield

            def emit_vproj(t):
                # V for key-tile t, both kv heads: [128 keys, 128] via 16 MMs.
                ps = psp.tile([P, P], f32, tag="ops", bufs=1)
                for h in range(HT):
                    nc.tensor.matmul(
                        ps,
                        xt_sb[:, h, t * P : (t + 1) * P],
                        wvt_sb[:, h, :],
                        start=(h == 0),
                        stop=(h == HT - 1),
                    )
                nc.vector.tensor_copy(
                    v2[:, t, :, 0:HD], ps.rearrange("p (g d) -> p g d", d=HD)
                )

            def oproj_chunk_gen(qb, qloc, oc, tag):
                # out[qb*512+qloc*128 :, oc*512 :] partial: 4 MMs + evict+DMA.
                ps = psp.tile([P, SQ], f32, tag=tag, bufs=1)
                for a in range(4):
                    nc.tensor.matmul(
                        ps,
                        ao_sb[:, qb % 2, a, qloc * P : (qloc + 1) * P],
                        wot_sb[:, a, oc * SQ : (oc + 1) * SQ],
                        start=(a == 0),
                        stop=(a == 3),
                    )
                    if a == 1:
                        yield
                ot = outp.tile([P, SQ], f32, tag="ot")
                nc.vector.tensor_copy(ot, ps)
                nc.gpsimd.dma_start(
                    out_d[
                        qb * SQ + qloc * P : qb * SQ + (qloc + 1) * P,
                        oc * SQ : (oc + 1) * SQ,
                    ],
                    ot,
                )
                yield

            # prologue compute: K.T block 0, V tile 0, Q(unit 0).
            for _ in kproj_gen(0):
                pass
            qt_box = []
            for _ in qproj_gen(0, qt_box):
                pass
            nc.sync.dma_start(
                wvt_sb[:], wvt_d.rearrange("(ht p) d -> p ht d", p=P)
            )
            emit_vproj(0)

            # bulk: x.T s-blocks 1..3, then Wo.T (needed from ~unit 4).
            for nb in range(1, 4):
                nc.sync.dma_start(
                    xt_sb[:, :, nb * SQ : (nb + 1) * SQ],
                    xt_r[:, :, nb * SQ : (nb + 1) * SQ],
                )
            nc.sync.dma_start(
                wot_sb[:], wot_d.rearrange("(a p) o -> p a o", p=P)
            )

            # ------------- interleave schedule -------------
            inserts = {}

            def sched(u, k, thunk):
                inserts.setdefault((u, k), []).append(thunk)

            # unit 0: K.T blocks 1..3 (block nb over iters 4nb-4..4nb-1) and
            # V tiles 1..15 (tile t at iter t).
            kgens = {nb: None for nb in range(1, 4)}

            def kproj_step(nb):
                if kgens[nb] is None:
                    kgens[nb] = kproj_gen(nb)
                next(kgens[nb], None)

            for nb in range(1, 4):
                for kk in range(4 * nb - 4, 4 * nb):
                    sched(0, kk, lambda nb=nb: kproj_step(nb))
            for t in range(1, KT):
                sched(0, t, lambda t=t: emit_vproj(t))

            # O-proj for q-block qb: 16 chunks x 2 gen-steps over units
            # 4(qb+1)..+3 at k=0..7 (qps idle there; alternate ops/qps).
            def wire_oproj(qb):
                chunks = [
                    (qloc, oc) for qloc in range(SQ // P) for oc in range(H // SQ)
                ]
                ci = 0
                slot_tags = ["ops", "qps", "ops", "ops"]
                for uu in range(4 * (qb + 1), 4 * (qb + 1) + 4):
                    for slot in range(4):
                        qloc, oc = chunks[ci]
                        tag = slot_tags[slot]
                        g = [None]

                        def step(qb=qb, qloc=qloc, oc=oc, tag=tag, g=g):
                            if g[0] is None:
                                g[0] = oproj_chunk_gen(qb, qloc, oc, tag)
                            next(g[0], None)

                        sched(uu, 2 * slot, step)
                        sched(uu, 2 * slot + 1, step)
                        ci += 1

            for qb in range(3):
                wire_oproj(qb)

            # ------------- flat software-pipelined unit stream -------------
            def emit_unit_eviction(u, oaccA, oaccB):
                qb, pr = u // 4, u % 4
                for half, oacc in ((0, oaccA), (64, oaccB)):
                    nc.vector.tensor_copy(
                        uo_sb[half : half + HD, pr, :], oacc[0:HD, :]
                    )
                    den_h = denp.tile([1, SQ], f32, tag="denh", bufs=4)
                    nc.vector.tensor_copy(den_h, oacc[HD : HD + 1, :])
                    rr = denp.tile([1, SQ], f32, tag="rr", bufs=4)
                    nc.vector.reciprocal_approx_fast(rr, den_h)
                    den_rb = denp.tile([P, SQ], f32, tag="denrb", bufs=3)
                    nc.sync.dma_start(
                        den_rb[half : half + HD, :],
                        rr[:, None, :].to_broadcast([1, HD, SQ]),
                    )
                    nc.vector.tensor_mul(
                        out=ao_sb[half : half + HD, qb % 2, pr, :],
                        in0=uo_sb[half : half + HD, pr, :],
                        in1=den_rb[half : half + HD, :],
                    )

            qgen = None
            prev_pt = None
            oacc_cur = None  # (oaccA, oaccB) of unit u
            oacc_prev = None
            for i in range(NIT):
                u, k = i // KT, i % KT
                qb, pr = u // 4, u % 4
                if k == 0:
                    qt = qt_box.pop(0)
                    qgen = None
                    oacc_prev = oacc_cur
                    oaccA = psp.tile([P, SQ], f32, tag="oacc", bufs=2)
                    oaccB = psp.tile([P, SQ], f32, tag="oacc", bufs=2)
                    oacc_cur = (oaccA, oaccB)
                lg = psp.tile([P, 2 * SQ], f32, tag="lg", bufs=2)
                nc.tensor.matmul(
                    lg[:, 0:SQ],
                    kt_sb[0:64, k * P : (k + 1) * P],
                    qt[0:64, :],
                    start=True,
                    stop=True,
                    tile_position=(0, 0),
                )
                nc.tensor.matmul(
                    lg[:, SQ : 2 * SQ],
                    kt_sb[64:128, k * P : (k + 1) * P],
                    qt[64:128, :],
                    start=True,
                    stop=True,
                    tile_position=(64, 0),
                )
                for thunk in inserts.get((u, k), ()):
                    thunk()
                if prev_pt is not None:
                    kk = (i - 1) % KT
                    oA, oB = oacc_cur if kk != KT - 1 else oacc_prev
                    nc.tensor.matmul(
                        oA,
                        v_sb[:, kk, 0:P],
                        prev_pt[:, 0:SQ],
                        start=(kk == 0),
                        stop=(kk == KT - 1),
                    )
                    nc.tensor.matmul(
                        oB,
                        v_sb[:, kk, HD + 1 : HD + 1 + P],
                        prev_pt[:, SQ : 2 * SQ],
                        start=(kk == 0),
                        stop=(kk == KT - 1),
                    )
                    if kk == KT - 1:
                        emit_unit_eviction(u - 1, oacc_prev[0], oacc_prev[1])
                if 6 <= k and u + 1 < NUNIT:
                    if qgen is None:
                        qgen = qproj_gen(u + 1, qt_box)
                    next(qgen, None)
                pt = ptp.tile([P, 2 * SQ], bf16, tag="pt")
                nc.scalar.activation(pt, lg, Exp, scale=SCALE)
                prev_pt = pt

            # final AV + eviction for unit 15
            kk = KT - 1
            nc.tensor.matmul(
                oacc_cur[0], v_sb[:, kk, 0:P], prev_pt[:, 0:SQ],
                start=False, stop=True,
            )
            nc.tensor.matmul(
                oacc_cur[1], v_sb[:, kk, HD + 1 : HD + 1 + P],
                prev_pt[:, SQ : 2 * SQ], start=False, stop=True,
            )
            emit_unit_eviction(NUNIT - 1, oacc_cur[0], oacc_cur[1])

            # ------------- tail: q-block 3 output projection -------------
            ci = 0
            for qloc in range(SQ // P):
                for oc in range(H // SQ):
                    tag = "ops" if ci % 2 == 0 else "qps"
                    for _ in oproj_chunk_gen(3, qloc, oc, tag):
                        pass
                    ci += 1

    nc.compile()
    _built_nc = nc
    return nc


def host_prep(x, Wq, bq, Wk, bk, Wv, bv, Wo, bo):
    """Returns the list of 8 per-core input maps."""
    import ml_dtypes

    bf = ml_dtypes.bfloat16
    x = np.asarray(x, np.float32)
    Wq = np.asarray(Wq, np.float32)
    Wk = np.asarray(Wk, np.float32)
    Wv = np.asarray(Wv, np.float32)
    Wo = np.asarray(Wo, np.float32)
    bq = np.asarray(bq, np.float32)
    bk = np.asarray(bk, np.float32)

    xts = [np.ascontiguousarray(x[b].T).astype(bf) for b in range(B)]

    per_g = []
    for g in range(4):
        rows = []
        for j in range(4):
            hA, hB = 8 * g + j, 8 * g + 4 + j
            rows += list(range(HD * hA, HD * hA + HD))
            rows += list(range(HD * hB, HD * hB + HD))
        rows = np.array(rows)
        kv = slice(P * g, P * g + P)
        per_g.append(
            {
                "wqt": np.ascontiguousarray(Wq[rows, :].T).astype(bf),
                "wkt": np.ascontiguousarray(Wk[kv, :].T).astype(bf),
                "wvt": np.ascontiguousarray(Wv[kv, :].T).astype(bf),
                "wot": np.ascontiguousarray(Wo[:, rows].T).astype(bf),
                "bqp": np.ascontiguousarray(bq[rows]),
                "bkp": np.ascontiguousarray(bk[kv]),
            }
        )

    in_maps = []
    for c in range(NCORES):
        b, g = c // 4, c % 4
        m = {"xt": xts[b]}
        m.update(per_g[g])
        in_maps.append(m)
    return in_maps


def host_corrections(out_full, Wv_bias, Wo, bo):
    """Add the bv/bo contributions (exact: softmax rows sum to 1)."""
    bv = np.asarray(Wv_bias, np.float32)
    bo = np.asarray(bo, np.float32)
    if np.any(bv):
        bv_full = np.repeat(
            np.asarray(bv).reshape(NKV, HD), NQ // NKV, axis=0
        ).reshape(H)
        out_full += (bv_full @ np.asarray(Wo, np.float32).T)[None, None, :]
    if np.any(bo):
        out_full += bo[None, None, :]
    return out_full


def kernel(x, Wq, bq, Wk, bk, Wv, bv, Wo, bo):
    global LAST_EXEC_NS, LAST_RESULT
    nc = build()
    in_maps = host_prep(x, Wq, bq, Wk, bk, Wv, bv, Wo, bo)

    from concourse.bass_utils import run_bass_kernel_spmd

    trace = bool(int(os.environ.get("KTRACE", "0")))
    res = run_bass_kernel_spmd(
        nc, in_maps, core_ids=list(range(NCORES)), trace=trace
    )
    LAST_RESULT = res
    LAST_EXEC_NS = res.exec_time_ns

    out = np.empty((B, S, H), np.float32)
    for b in range(B):
        acc = res.results[4 * b]["out"].astype(np.float32)
        for g in range(1, 4):
            acc = acc + res.results[4 * b + g]["out"]
        out[b] = acc
    out = host_corrections(out, bv, Wo, bo)
    return out
